# revision 4
# baseline (speedup 1.0000x reference)
"""Q4_0-quantized linear: y = x @ dequant(W).T on 8 Trainium2 cores.

Column-parallel (tensor-parallel) sharding: W's 11008 output rows are split
into 8 shards of 1376; each core computes x @ W_shard.T for the full batch
and shards are concatenated on the host.

Host-side prep is pure layout repacking (no arithmetic on values):
  - x [4,2048,4096] fp16 -> xT [4096, 8192] contiguous (contraction dim on
    SBUF partitions for the PE matmul).
  - packed int4 nibbles -> sign-extended int8 planes, laid out per k-tile as
    q8[t, j, o]: partition j of k-tile t holds the int weight for input
    feature k = 128*t + j of output row o.
  - per-group fp16 scales expanded to the same [t, 128, o] layout.

Device (per core, identical SPMD program):
  Phase A: wdT[:, t] = q8[t] * sc[t]  (int8 x fp16 -> fp16, VectorE), giving
           the dequantized W_shard^T resident in SBUF as 32 k-tiles
           [128, 1376] -- 11.3 MB.
  Phase B: for each 128-row tile of x: 32 (k) x 3 (n-chunk) matmuls
           accumulate x_tile @ W_shard.T into PSUM [128, 1376] fp32,
           copy to fp16, DMA out.
"""

import numpy as np

import concourse.bass as bass
import concourse.bacc as bacc
import concourse.mybir as mybir
from concourse import tile
from concourse.bass_utils import run_bass_kernel_spmd

GROUP = 64
OUT_F, IN_F = 11008, 4096
B, S = 4, 2048
M = B * S                      # 8192 rows of x
NCORES = 8
N_SHARD = OUT_F // NCORES      # 1376 output features per core
KT = IN_F // 128               # 32 k-tiles of 128


def build_program(m_rows=M, n_shard=N_SHARD, kt=KT):
    """Build the single-core Bass program (SPMD: same program on all cores)."""
    nc = bacc.Bacc(
        "TRN2", target_bir_lowering=False, debug=False, num_devices=NCORES
    )
    dt = mybir.dt

    xT = nc.dram_tensor("xT", [kt * 128, m_rows], dt.float16, kind="ExternalInput")
    q8 = nc.dram_tensor("q8", [kt, 128, n_shard], dt.int8, kind="ExternalInput")
    sc = nc.dram_tensor("sc", [kt, 128, n_shard], dt.float16, kind="ExternalInput")
    y = nc.dram_tensor("y", [m_rows, n_shard], dt.float16, kind="ExternalOutput")

    # n-chunks of <=512 fp32 so each matmul stays inside one PSUM bank
    n_chunks = []
    n0 = 0
    while n0 < n_shard:
        w = min(512, n_shard - n0)
        n_chunks.append((n0, w))
        n0 += w

    xT_tiles = xT.rearrange("(t p) m -> p t m", p=128)  # [128, kt, m_rows]

    with tile.TileContext(nc) as tc:
        with (
            tc.tile_pool(name="wres", bufs=1) as wres,
            tc.tile_pool(name="dq", bufs=3) as dq,
            tc.tile_pool(name="xp", bufs=3) as xp,
            tc.tile_pool(name="op", bufs=3) as op,
            tc.tile_pool(name="ps", bufs=2, space="PSUM") as ps,
        ):
            # resident dequantized W^T: k-tile t lives at free offset t*n_shard
            wdT = wres.tile([128, kt * n_shard], dt.float16)

            # ---- Phase A: dequantize ----
            for t in range(kt):
                qt = dq.tile([128, n_shard], dt.int8, tag="qt")
                nc.sync.dma_start(qt[:], q8[t])
                sct = dq.tile([128, n_shard], dt.float16, tag="sct")
                nc.sync.dma_start(sct[:], sc[t])
                nc.vector.tensor_tensor(
                    wdT[:, t * n_shard : (t + 1) * n_shard],
                    qt[:],
                    sct[:],
                    mybir.AluOpType.mult,
                )

            # ---- Phase B: GEMM ----
            for mi in range(m_rows // 128):
                xm = xp.tile([128, kt * 128], dt.float16, tag="xm")
                nc.sync.dma_start(
                    xm[:].rearrange("p (t j) -> p t j", t=kt),
                    xT_tiles[:, :, mi * 128 : (mi + 1) * 128],
                )
                psum = ps.tile([128, n_shard], dt.float32, tag="psum")
                for k in range(kt):
                    for (c0, cw) in n_chunks:
                        nc.tensor.matmul(
                            psum[:, c0 : c0 + cw],
                            xm[:, k * 128 : (k + 1) * 128],
                            wdT[:, k * n_shard + c0 : k * n_shard + c0 + cw],
                            start=(k == 0),
                            stop=(k == kt - 1),
                        )
                out_sb = op.tile([128, n_shard], dt.float16, tag="out")
                nc.any.tensor_copy(out_sb[:], psum[:])
                nc.sync.dma_start(y[mi * 128 : (mi + 1) * 128, :], out_sb[:])

    nc.compile()
    return nc


def prep_inputs(x, linear_w, linear_s, n_shard=N_SHARD, kt=KT, ncores=NCORES):
    """Host-side layout repacking -> per-core input maps."""
    x2 = np.asarray(x, dtype=np.float16).reshape(-1, IN_F)
    xT = np.ascontiguousarray(x2.T)  # [IN_F, M]

    w = np.asarray(linear_w, dtype=np.int8)       # [OUT_F*32, 64] packed
    s = np.asarray(linear_s, dtype=np.float16)    # [OUT_F*64, 1]

    # unpack nibbles (sign-extending) -> per-row int values [OUT_F, 32, 2, 64]
    msb = (w >> 4).reshape(OUT_F, 32, 64)
    lsb = (w.astype(np.int8) << 4 >> 4).reshape(OUT_F, 32, 64)
    # q[o, t, j]: j<64 -> group 2t value j (msb), j>=64 -> group 2t+1 (lsb)
    q = np.concatenate([msb, lsb], axis=2)        # [OUT_F, 32, 128]
    sg = s.reshape(OUT_F, GROUP)                  # scale of (o, g)
    # sc_exp[o, t, j] = scale(o, 2t) for j<64 else scale(o, 2t+1)
    sc_exp = np.repeat(sg.reshape(OUT_F, 32, 2), GROUP, axis=2)  # [OUT_F, 32, 128]

    in_maps = []
    for c in range(ncores):
        o0 = c * n_shard
        qc = np.ascontiguousarray(q[o0 : o0 + n_shard].transpose(1, 2, 0))       # [32,128,n]
        scc = np.ascontiguousarray(sc_exp[o0 : o0 + n_shard].transpose(1, 2, 0))  # [32,128,n]
        in_maps.append({"xT": xT, "q8": qc, "sc": scc})
    return in_maps


_CACHED = {}


def kernel(x, linear_w, linear_s):
    if "nc" not in _CACHED:
        _CACHED["nc"] = build_program()
    nc = _CACHED["nc"]
    in_maps = prep_inputs(x, linear_w, linear_s)
    res = run_bass_kernel_spmd(nc, in_maps, list(range(NCORES)))
    y = np.concatenate([r["y"] for r in res.results], axis=1)  # [M, OUT_F]
    return y.reshape(B, S, OUT_F).astype(np.float16)


# revision 11
# speedup vs baseline: 54.7014x; 54.7014x over previous
"""Q4_0-quantized linear: y = x @ dequant(W).T on 8 Trainium2 cores.

Column-parallel (tensor-parallel) sharding: W's 11008 output rows are split
into 8 shards of 1376; each core computes x @ W_shard.T for the full batch
and shards are concatenated on the host.

Host-side prep is pure layout repacking (no arithmetic on values):
  - x [4,2048,4096] fp16 -> xT [4096, 8192] contiguous (contraction dim on
    SBUF partitions for the PE matmul).
  - packed int4 nibbles -> sign-extended int8 planes, laid out per k-tile as
    q8[t, j, o]: partition j of k-tile t holds the int weight for input
    feature k = 128*t + j of output row o.
  - per-group fp16 scales expanded to the same [t, 128, o] layout.

Device (per core, identical SPMD program):
  Phase A: wdT[:, t] = q8[t] * sc[t]  (int8 x fp16 -> fp16, VectorE), giving
           the dequantized W_shard^T resident in SBUF as 32 k-tiles
           [128, 1376] -- 11.3 MB.
  Phase B: for each 128-row tile of x: 32 (k) x 3 (n-chunk) matmuls
           accumulate x_tile @ W_shard.T into PSUM [128, 1376] fp32,
           copy to fp16, DMA out.
"""

import numpy as np

import concourse.bass as bass
import concourse.bacc as bacc
import concourse.mybir as mybir
from concourse import tile
from concourse.bass_utils import run_bass_kernel_spmd

GROUP = 64
OUT_F, IN_F = 11008, 4096
B, S = 4, 2048
M = B * S                      # 8192 rows of x
NCORES = 8
N_SHARD = OUT_F // NCORES      # 1376 output features per core
KT = IN_F // 128               # 32 k-tiles of 128


def build_program(m_rows=M, n_shard=N_SHARD, kt=KT, repeat=1):
    """Build the single-core Bass program (SPMD: same program on all cores).

    repeat>1 wraps the whole kernel in an on-device loop — used only for
    timing (wall-clock deltas between repeat counts cancel dispatch latency).
    """
    nc = bacc.Bacc(
        "TRN2", target_bir_lowering=False, debug=False, num_devices=NCORES
    )
    dt = mybir.dt

    # xr[mi, p, t*128+j] = x[mi*128 + j, t*128 + p]: per-m-tile x^T, dense
    xr = nc.dram_tensor(
        "xr", [m_rows // 128, 128, kt * 128], dt.float16, kind="ExternalInput"
    )
    q8 = nc.dram_tensor("q8", [kt, 128, n_shard], dt.int8, kind="ExternalInput")
    sc = nc.dram_tensor("sc", [kt, 128, n_shard], dt.float16, kind="ExternalInput")
    y = nc.dram_tensor("y", [m_rows, n_shard], dt.float16, kind="ExternalOutput")

    # n-chunks of <=512 fp32 so each matmul stays inside one PSUM bank
    n_chunks = []
    n0 = 0
    while n0 < n_shard:
        w = min(512, n_shard - n0)
        n_chunks.append((n0, w))
        n0 += w

    with tile.TileContext(nc) as tc:
        with (
            tc.tile_pool(name="wres", bufs=1) as wres,
            tc.tile_pool(name="dq", bufs=3) as dq,
            tc.tile_pool(name="xp", bufs=3) as xp,
            tc.tile_pool(name="op", bufs=3) as op,
            tc.tile_pool(name="ps", bufs=2, space="PSUM") as ps,
        ):

            def body():
                # resident dequantized W^T: k-tile t at free offset t*n_shard
                wdT = wres.tile([128, kt * n_shard], dt.float16, tag="wdT")

                # ---- Phase A: dequantize ----
                for t in range(kt):
                    qt = dq.tile([128, n_shard], dt.int8, tag="qt")
                    nc.sync.dma_start(qt[:], q8[t])
                    sct = dq.tile([128, n_shard], dt.float16, tag="sct")
                    nc.sync.dma_start(sct[:], sc[t])
                    nc.vector.tensor_tensor(
                        wdT[:, t * n_shard : (t + 1) * n_shard],
                        qt[:],
                        sct[:],
                        mybir.AluOpType.mult,
                    )

                # ---- Phase B: GEMM ----
                for mi in range(m_rows // 128):
                    xm = xp.tile([128, kt * 128], dt.float16, tag="xm")
                    nc.sync.dma_start(xm[:], xr[mi])
                    psum = ps.tile([128, n_shard], dt.float32, tag="psum")
                    for k in range(kt):
                        for (c0, cw) in n_chunks:
                            nc.tensor.matmul(
                                psum[:, c0 : c0 + cw],
                                xm[:, k * 128 : (k + 1) * 128],
                                wdT[:, k * n_shard + c0 : k * n_shard + c0 + cw],
                                start=(k == 0),
                                stop=(k == kt - 1),
                            )
                    out_sb = op.tile([128, n_shard], dt.float16, tag="out")
                    nc.any.tensor_copy(out_sb[:], psum[:])
                    nc.sync.dma_start(y[mi * 128 : (mi + 1) * 128, :], out_sb[:])

            if repeat > 1:
                with tc.For_i(0, repeat, 1):
                    body()
            else:
                body()

    nc.compile()
    return nc


def prep_inputs(x, linear_w, linear_s, n_shard=N_SHARD, kt=KT, ncores=NCORES):
    """Host-side layout repacking -> per-core input maps."""
    x2 = np.asarray(x, dtype=np.float16).reshape(-1, IN_F)
    # [mi, p, t*128+j] = x[128*mi + j, 128*t + p] — per-m-tile transposed, dense
    xr = np.ascontiguousarray(
        x2.reshape(M // 128, 128, KT, 128).transpose(0, 3, 2, 1)
    ).reshape(M // 128, 128, IN_F)

    w = np.asarray(linear_w, dtype=np.int8)       # [OUT_F*32, 64] packed
    s = np.asarray(linear_s, dtype=np.float16)    # [OUT_F*64, 1]

    # unpack nibbles (sign-extending) -> per-row int values [OUT_F, 32, 2, 64]
    msb = (w >> 4).reshape(OUT_F, 32, 64)
    lsb = (w.astype(np.int8) << 4 >> 4).reshape(OUT_F, 32, 64)
    # q[o, t, j]: j<64 -> group 2t value j (msb), j>=64 -> group 2t+1 (lsb)
    q = np.concatenate([msb, lsb], axis=2)        # [OUT_F, 32, 128]
    sg = s.reshape(OUT_F, GROUP)                  # scale of (o, g)
    # sc_exp[o, t, j] = scale(o, 2t) for j<64 else scale(o, 2t+1)
    sc_exp = np.repeat(sg.reshape(OUT_F, 32, 2), GROUP, axis=2)  # [OUT_F, 32, 128]

    in_maps = []
    for c in range(ncores):
        o0 = c * n_shard
        qc = np.ascontiguousarray(q[o0 : o0 + n_shard].transpose(1, 2, 0))       # [32,128,n]
        scc = np.ascontiguousarray(sc_exp[o0 : o0 + n_shard].transpose(1, 2, 0))  # [32,128,n]
        in_maps.append({"xr": xr, "q8": qc, "sc": scc})
    return in_maps


_CACHED = {}


def kernel(x, linear_w, linear_s):
    if "nc" not in _CACHED:
        _CACHED["nc"] = build_program()
    nc = _CACHED["nc"]
    in_maps = prep_inputs(x, linear_w, linear_s)
    res = run_bass_kernel_spmd(nc, in_maps, list(range(NCORES)))
    y = np.concatenate([r["y"] for r in res.results], axis=1)  # [M, OUT_F]
    return y.reshape(B, S, OUT_F).astype(np.float16)
